# revision 14
# baseline (speedup 1.0000x reference)
"""Two-layer GCN (PyG GCNConv x2 + ReLU) on 8 Trainium2 NeuronCores via Bass.

Formulation: GCN aggregation is linear row-mixing, so for each layer
    conv(H) = A_hat @ H @ W + b      (A_hat includes self-loops, sym-norm)
and we aggregate FIRST, then matmul:
    z   = relu(A_hat @ x @ W1 + b1)
    out = A_hat @ z @ W2 + b2

Sharding: nodes split into 8 row-slabs (2560 padded rows each). Each core
aggregates + matmuls its own dst rows. The only communication is one
AllGather of z (bf16) so every core can gather arbitrary src rows for L2.

Aggregation on device: edges are bucketed by 128-dst-node "supertiles",
padded to B 128-edge blocks per supertile. For each block, the gathered
src-rows tile G [128 edges, C] and a host-built selection matrix
S^T [128 edges, 128 dst] (entry [e, dstl] = norm(e)) produce
    agg^T[ch_tile, dst] += G[:, ch_tile].T @ S^T     (PSUM accumulate)
which directly yields agg^T as matmul lhsT for the subsequent @W.

L1 gathers are free: x is a kernel input, so the edge-ordered gather table
xe is prebuilt on host and streamed sequentially. L2 gathers rows of z_full
via indirect DMA (the only runtime gather).
"""

import numpy as np
import ml_dtypes

N_NODES = 20000
IN_CH = 256
HID_CH = 512
N_CORES = 8
P = 128
NPAD = 20480            # 160 supertiles of 128
NSUP = NPAD // P        # 160
SUP_PER_CORE = NSUP // N_CORES  # 20
ROWS_PER_CORE = NPAD // N_CORES  # 2560

bf16 = ml_dtypes.bfloat16

TRACE = False           # set by test harness for HW profiling
DEBUG_Z = False         # add a z_full dump output (debug only)
LAST_RESULTS = None     # BassKernelResults of the last device run

_COMPILED = {}          # B -> (nc, input names metadata)


def _preprocess(x, edge_index):
    """Build per-core gather/selection tables. Returns dict of host arrays."""
    src = edge_index[0].astype(np.int64)
    dst = edge_index[1].astype(np.int64)
    n = N_NODES

    deg = np.bincount(dst, minlength=n).astype(np.float32) + 1.0
    dinv = 1.0 / np.sqrt(deg)

    # edge list + self-loops, weights w = dinv[src]*dinv[dst] (self: dinv^2)
    allsrc = np.concatenate([src, np.arange(n, dtype=np.int64)])
    alldst = np.concatenate([dst, np.arange(n, dtype=np.int64)])
    allw = np.concatenate([dinv[src] * dinv[dst], dinv * dinv]).astype(np.float32)

    order = np.argsort(alldst, kind="stable")
    s_src = allsrc[order]
    s_dst = alldst[order]
    s_w = allw[order]

    sup = (s_dst >> 7).astype(np.int64)          # supertile per edge
    cnt = np.bincount(sup, minlength=NSUP)
    B = int(np.ceil(cnt.max() / P))              # uniform blocks per supertile
    CAP = B * P

    starts = np.zeros(NSUP, np.int64)
    starts[1:] = np.cumsum(cnt)[:-1]
    slot = np.arange(len(s_dst)) - starts[sup]   # slot within supertile
    gslot = sup * CAP + slot                     # global padded slot

    esrc = np.zeros(NSUP * CAP, np.int32)        # padded: src=0, w=0
    esrc[gslot] = s_src.astype(np.int32)
    edstl = np.zeros(NSUP * CAP, np.int64)
    edstl[gslot] = s_dst & 127
    ew = np.zeros(NSUP * CAP, np.float32)
    ew[gslot] = s_w

    # S^T blocks: [NSUP, slot, dstl] -> DMA layout [NSUP, p, b*128+dstl]
    st = np.zeros((NSUP * CAP, P), bf16)
    st[np.arange(NSUP * CAP), edstl] = ew.astype(bf16)
    st = (
        st.reshape(NSUP, B, P, P)
        .transpose(0, 2, 1, 3)                   # [sup, p, b, dstl]
        .reshape(NSUP, P, B * P)
    )

    # L2 gather indices: [NSUP, p, b]
    zidx = esrc.reshape(NSUP, B, P).transpose(0, 2, 1).copy()

    # L1 pregathered edge-ordered x: [NSUP, p, b*IN_CH]
    xbf = np.ascontiguousarray(x.astype(bf16))
    xe = (
        xbf[esrc.reshape(NSUP, B, P)]            # [sup, b, p, IN_CH]
        .transpose(0, 2, 1, 3)
        .reshape(NSUP, P, B * IN_CH)
    )

    return {"B": B, "st": st, "zidx": zidx, "xe": xe}


def _build_program(B, has_bias):
    import concourse.bass as bass
    import concourse.mybir as mybir
    import concourse.tile as tile
    from concourse.bacc import Bacc

    dt = mybir.dt
    nc = Bacc("TRN2", target_bir_lowering=False, debug=False, num_devices=N_CORES)

    t_xe = nc.dram_tensor("xe", [SUP_PER_CORE, P, B * IN_CH], dt.bfloat16,
                          kind="ExternalInput")
    t_st = nc.dram_tensor("st", [SUP_PER_CORE, P, B * P], dt.bfloat16,
                          kind="ExternalInput")
    t_zidx = nc.dram_tensor("zidx", [SUP_PER_CORE, P, B], dt.int32,
                            kind="ExternalInput")
    t_w1 = nc.dram_tensor("w1", [2, P, HID_CH], dt.bfloat16, kind="ExternalInput")
    t_w2 = nc.dram_tensor("w2", [4, P, HID_CH], dt.bfloat16, kind="ExternalInput")
    if has_bias:
        t_b1 = nc.dram_tensor("b1b", [P, HID_CH], dt.float32, kind="ExternalInput")
        t_b2 = nc.dram_tensor("b2b", [P, HID_CH], dt.float32, kind="ExternalInput")
    t_out = nc.dram_tensor("out", [ROWS_PER_CORE, HID_CH], dt.float32,
                           kind="ExternalOutput")
    t_zdbg = None
    if DEBUG_Z:
        t_zdbg = nc.dram_tensor("zdbg", [NPAD, HID_CH], dt.bfloat16,
                                kind="ExternalOutput")

    K1 = IN_CH // P   # 2 ch tiles in L1
    K2 = HID_CH // P  # 4 ch tiles in L2

    with tile.TileContext(nc) as tc:
        with (
            tc.tile_pool(name="dram", bufs=1, space="DRAM") as dram,
            tc.tile_pool(name="const", bufs=1) as cpool,
            tc.tile_pool(name="work", bufs=3) as pool,
            tc.tile_pool(name="stres", bufs=1) as stpool,
        ):
            z_slice = dram.tile([ROWS_PER_CORE, HID_CH], dt.bfloat16, name="z_slice")
            z_full = dram.tile([NPAD, HID_CH], dt.bfloat16, name="z_full",
                               addr_space="Shared")

            w1_t = cpool.tile([P, K1 * HID_CH], dt.bfloat16, name="w1_t")
            for m in range(K1):
                nc.sync.dma_start(out=w1_t[:, m * HID_CH:(m + 1) * HID_CH],
                                  in_=t_w1[m])
            w2_t = cpool.tile([P, K2 * HID_CH], dt.bfloat16, name="w2_t")
            for m in range(K2):
                nc.sync.dma_start(out=w2_t[:, m * HID_CH:(m + 1) * HID_CH],
                                  in_=t_w2[m])
            if has_bias:
                b1_t = cpool.tile([P, HID_CH], dt.float32, name="b1_t")
                nc.sync.dma_start(out=b1_t[:], in_=t_b1[:])
                b2_t = cpool.tile([P, HID_CH], dt.float32, name="b2_t")
                nc.sync.dma_start(out=b2_t[:], in_=t_b2[:])

            st_res = []  # S^T kept resident in SBUF; reused by L2
            # ---------------- Layer 1 ----------------
            with tc.tile_pool(name="psum1", bufs=2, space="PSUM") as psum1:
                for s in range(SUP_PER_CORE):
                    xe_t = pool.tile([P, B * IN_CH], dt.bfloat16, tag="xe",
                                     name=f"xe{s}")
                    nc.sync.dma_start(out=xe_t[:], in_=t_xe[s])
                    st_t = stpool.tile([P, B * P], dt.bfloat16, tag=f"st{s}",
                                       name=f"st{s}")
                    nc.sync.dma_start(out=st_t[:], in_=t_st[s])
                    st_res.append(st_t)

                    a1 = [
                        psum1.tile([P, P], dt.float32, tag=f"a1_{m}",
                                   name=f"a1_{s}_{m}")
                        for m in range(K1)
                    ]
                    for b in range(B):
                        for m in range(K1):
                            nc.tensor.matmul(
                                out=a1[m][:],
                                lhsT=xe_t[:, b * IN_CH + m * P:
                                          b * IN_CH + (m + 1) * P],
                                rhs=st_t[:, b * P:(b + 1) * P],
                                start=(b == 0),
                                stop=(b == B - 1),
                            )
                    a1s = pool.tile([P, K1 * P], dt.bfloat16, tag="a1s",
                                    name=f"a1s{s}")
                    for m in range(K1):
                        nc.vector.tensor_copy(out=a1s[:, m * P:(m + 1) * P],
                                              in_=a1[m][:])

                    zp = psum1.tile([P, HID_CH], dt.float32, tag="zp",
                                    name=f"zp{s}")
                    for m in range(K1):
                        nc.tensor.matmul(
                            out=zp[:],
                            lhsT=a1s[:, m * P:(m + 1) * P],
                            rhs=w1_t[:, m * HID_CH:(m + 1) * HID_CH],
                            start=(m == 0),
                            stop=(m == K1 - 1),
                        )
                    z_t = pool.tile([P, HID_CH], dt.bfloat16, tag="z",
                                    name=f"z{s}")
                    if has_bias:
                        nc.vector.tensor_add(out=zp[:], in0=zp[:], in1=b1_t[:])
                    nc.scalar.activation(out=z_t[:], in_=zp[:],
                                         func=mybir.ActivationFunctionType.Relu)
                    nc.sync.dma_start(out=z_slice[s * P:(s + 1) * P, :], in_=z_t[:])

            # ---------------- AllGather z ----------------
            nc.gpsimd.collective_compute(
                "AllGather",
                mybir.AluOpType.bypass,
                replica_groups=[list(range(N_CORES))],
                ins=[z_slice.opt()],
                outs=[z_full.opt()],
            )
            if DEBUG_Z:
                nc.sync.dma_start(out=t_zdbg[:], in_=z_full[:])

            # ---------------- Layer 2 ----------------
            with tc.tile_pool(name="psum2", bufs=1, space="PSUM") as psum2:
                for s in range(SUP_PER_CORE):
                    zidx_t = pool.tile([P, B], dt.int32, tag="zidx",
                                       name=f"zidx{s}")
                    nc.sync.dma_start(out=zidx_t[:], in_=t_zidx[s])
                    g_t = pool.tile([P, B * HID_CH], dt.bfloat16, tag="g",
                                    name=f"g{s}")
                    for b in range(B):
                        nc.gpsimd.indirect_dma_start(
                            out=g_t[:, b * HID_CH:(b + 1) * HID_CH],
                            out_offset=None,
                            in_=z_full[:],
                            in_offset=bass.IndirectOffsetOnAxis(
                                ap=zidx_t[:, b:b + 1], axis=0
                            ),
                        )
                    st_t = st_res[s]
                    a2 = [
                        psum2.tile([P, P], dt.float32, tag=f"a2_{m}",
                                   name=f"a2_{s}_{m}")
                        for m in range(K2)
                    ]
                    for b in range(B):
                        for m in range(K2):
                            nc.tensor.matmul(
                                out=a2[m][:],
                                lhsT=g_t[:, b * HID_CH + m * P:
                                         b * HID_CH + (m + 1) * P],
                                rhs=st_t[:, b * P:(b + 1) * P],
                                start=(b == 0),
                                stop=(b == B - 1),
                            )
                    a2s = pool.tile([P, K2 * P], dt.bfloat16, tag="a2s",
                                    name=f"a2s{s}")
                    for m in range(K2):
                        nc.vector.tensor_copy(out=a2s[:, m * P:(m + 1) * P],
                                              in_=a2[m][:])

                    op = psum2.tile([P, HID_CH], dt.float32, tag="op",
                                    name=f"op{s}")
                    for m in range(K2):
                        nc.tensor.matmul(
                            out=op[:],
                            lhsT=a2s[:, m * P:(m + 1) * P],
                            rhs=w2_t[:, m * HID_CH:(m + 1) * HID_CH],
                            start=(m == 0),
                            stop=(m == K2 - 1),
                        )
                    o_t = pool.tile([P, HID_CH], dt.float32, tag="o",
                                    name=f"o{s}")
                    if has_bias:
                        nc.vector.tensor_add(out=o_t[:], in0=op[:], in1=b2_t[:])
                    else:
                        nc.vector.tensor_copy(out=o_t[:], in_=op[:])
                    nc.sync.dma_start(out=t_out[s * P:(s + 1) * P, :], in_=o_t[:])

    nc.compile()
    return nc


def kernel(x, edge_index, W1, b1, W2, b2):
    global LAST_RESULTS
    from concourse import bass_utils

    x = np.asarray(x, np.float32)
    edge_index = np.asarray(edge_index)
    W1 = np.asarray(W1, np.float32)
    b1 = np.asarray(b1, np.float32)
    W2 = np.asarray(W2, np.float32)
    b2 = np.asarray(b2, np.float32)

    prep = _preprocess(x, edge_index)
    B = prep["B"]
    has_bias = bool(np.any(b1) or np.any(b2))

    key = (B, has_bias, DEBUG_Z)
    if key not in _COMPILED:
        _COMPILED[key] = _build_program(B, has_bias)
    nc = _COMPILED[key]

    w1_in = np.ascontiguousarray(
        W1.astype(bf16).reshape(2, P, HID_CH)
    )
    w2_in = np.ascontiguousarray(
        W2.astype(bf16).reshape(4, P, HID_CH)
    )

    in_maps = []
    for c in range(N_CORES):
        s0, s1 = c * SUP_PER_CORE, (c + 1) * SUP_PER_CORE
        m = {
            "xe": np.ascontiguousarray(prep["xe"][s0:s1]),
            "st": np.ascontiguousarray(prep["st"][s0:s1]),
            "zidx": np.ascontiguousarray(prep["zidx"][s0:s1]),
            "w1": w1_in,
            "w2": w2_in,
        }
        if has_bias:
            m["b1b"] = np.tile(b1.astype(np.float32)[None, :], (P, 1))
            m["b2b"] = np.tile(b2.astype(np.float32)[None, :], (P, 1))
        in_maps.append(m)

    res = bass_utils.run_bass_kernel_spmd(
        nc, in_maps, core_ids=list(range(N_CORES)), trace=TRACE,
    )
    LAST_RESULTS = res

    out = np.concatenate([res.results[c]["out"] for c in range(N_CORES)], axis=0)
    return np.ascontiguousarray(out[:N_NODES]).astype(np.float32)


# revision 20
# speedup vs baseline: 1.0376x; 1.0376x over previous
"""Two-layer GCN (PyG GCNConv x2 + ReLU) on 8 Trainium2 NeuronCores via Bass.

Formulation: GCN aggregation is linear row-mixing, so for each layer
    conv(H) = A_hat @ H @ W + b      (A_hat includes self-loops, sym-norm)
and we aggregate FIRST, then matmul:
    z   = relu(A_hat @ x @ W1 + b1)
    out = A_hat @ z @ W2 + b2

Sharding: nodes split into 8 row-slabs (2560 padded rows each). Each core
aggregates + matmuls its own dst rows. The only communication is one
AllGather of z (bf16) so every core can gather arbitrary src rows for L2.

Aggregation on device: edges are bucketed by 128-dst-node "supertiles",
padded to B 128-edge blocks per supertile. For each block, the gathered
src-rows tile G [128 edges, C] and a host-built selection matrix
S^T [128 edges, 128 dst] (entry [e, dstl] = norm(e)) produce
    agg^T[ch_tile, dst] += G[:, ch_tile].T @ S^T     (PSUM accumulate)
which directly yields agg^T as matmul lhsT for the subsequent @W.

L1 gathers are free: x is a kernel input, so the edge-ordered gather table
xe is prebuilt on host and streamed sequentially. L2 gathers rows of z_full
via indirect DMA (the only runtime gather).
"""

import numpy as np
import ml_dtypes

N_NODES = 20000
IN_CH = 256
HID_CH = 512
N_CORES = 8
P = 128
NPAD = 20480            # 160 supertiles of 128
NSUP = NPAD // P        # 160
SUP_PER_CORE = NSUP // N_CORES  # 20
ROWS_PER_CORE = NPAD // N_CORES  # 2560

bf16 = ml_dtypes.bfloat16

TRACE = False           # set by test harness for HW profiling
DEBUG_Z = False         # add a z_full dump output (debug only)
LAST_RESULTS = None     # BassKernelResults of the last device run

_COMPILED = {}          # B -> (nc, input names metadata)


def _preprocess(x, edge_index):
    """Build per-core gather/selection tables. Returns dict of host arrays."""
    src = edge_index[0].astype(np.int64)
    dst = edge_index[1].astype(np.int64)
    n = N_NODES

    deg = np.bincount(dst, minlength=n).astype(np.float32) + 1.0
    dinv = 1.0 / np.sqrt(deg)

    # edge list + self-loops, weights w = dinv[src]*dinv[dst] (self: dinv^2)
    allsrc = np.concatenate([src, np.arange(n, dtype=np.int64)])
    alldst = np.concatenate([dst, np.arange(n, dtype=np.int64)])
    allw = np.concatenate([dinv[src] * dinv[dst], dinv * dinv]).astype(np.float32)

    order = np.argsort(alldst, kind="stable")
    s_src = allsrc[order]
    s_dst = alldst[order]
    s_w = allw[order]

    sup = (s_dst >> 7).astype(np.int64)          # supertile per edge
    cnt = np.bincount(sup, minlength=NSUP)
    B = int(np.ceil(cnt.max() / P))              # uniform blocks per supertile
    CAP = B * P

    starts = np.zeros(NSUP, np.int64)
    starts[1:] = np.cumsum(cnt)[:-1]
    slot = np.arange(len(s_dst)) - starts[sup]   # slot within supertile
    gslot = sup * CAP + slot                     # global padded slot

    esrc = np.zeros(NSUP * CAP, np.int32)        # padded: src=0, w=0
    esrc[gslot] = s_src.astype(np.int32)
    edstl = np.zeros(NSUP * CAP, np.int64)
    edstl[gslot] = s_dst & 127
    ew = np.zeros(NSUP * CAP, np.float32)
    ew[gslot] = s_w

    # S^T blocks: [NSUP, slot, dstl] -> DMA layout [NSUP, p, b*128+dstl]
    st = np.zeros((NSUP * CAP, P), bf16)
    st[np.arange(NSUP * CAP), edstl] = ew.astype(bf16)
    st = (
        st.reshape(NSUP, B, P, P)
        .transpose(0, 2, 1, 3)                   # [sup, p, b, dstl]
        .reshape(NSUP, P, B * P)
    )

    # L2 gather indices: [NSUP, p, b]
    zidx = esrc.reshape(NSUP, B, P).transpose(0, 2, 1).copy()

    # L1 pregathered edge-ordered x: [NSUP, p, b*IN_CH]
    xbf = np.ascontiguousarray(x.astype(bf16))
    xe = (
        xbf[esrc.reshape(NSUP, B, P)]            # [sup, b, p, IN_CH]
        .transpose(0, 2, 1, 3)
        .reshape(NSUP, P, B * IN_CH)
    )

    return {"B": B, "st": st, "zidx": zidx, "xe": xe}


def _build_program(B, has_bias):
    import concourse.bass as bass
    import concourse.mybir as mybir
    import concourse.tile as tile
    from concourse.bacc import Bacc
    from concourse.masks import make_identity

    dt = mybir.dt
    nc = Bacc("TRN2", target_bir_lowering=False, debug=False, num_devices=N_CORES)

    t_xe = nc.dram_tensor("xe", [SUP_PER_CORE, P, B * IN_CH], dt.bfloat16,
                          kind="ExternalInput")
    t_st = nc.dram_tensor("st", [SUP_PER_CORE, P, B * P], dt.bfloat16,
                          kind="ExternalInput")
    t_zidx = nc.dram_tensor("zidx", [SUP_PER_CORE, P, B], dt.int32,
                            kind="ExternalInput")
    t_w1 = nc.dram_tensor("w1", [2, P, HID_CH], dt.bfloat16, kind="ExternalInput")
    t_w2 = nc.dram_tensor("w2", [4, P, HID_CH], dt.bfloat16, kind="ExternalInput")
    if has_bias:
        t_b1 = nc.dram_tensor("b1b", [P, HID_CH], dt.float32, kind="ExternalInput")
        t_b2 = nc.dram_tensor("b2b", [P, HID_CH], dt.float32, kind="ExternalInput")
    t_out = nc.dram_tensor("out", [ROWS_PER_CORE, HID_CH], dt.float32,
                           kind="ExternalOutput")
    t_zdbg = None
    if DEBUG_Z:
        t_zdbg = nc.dram_tensor("zdbg", [NPAD, HID_CH], dt.bfloat16,
                                kind="ExternalOutput")

    K1 = IN_CH // P   # 2 ch tiles in L1
    K2 = HID_CH // P  # 4 ch tiles in L2

    with tile.TileContext(nc) as tc:
        with (
            tc.tile_pool(name="dram", bufs=1, space="DRAM") as dram,
            tc.tile_pool(name="const", bufs=1) as cpool,
            tc.tile_pool(name="work", bufs=3) as pool,
            tc.tile_pool(name="stres", bufs=1) as stpool,
        ):
            z_slice = dram.tile([ROWS_PER_CORE, HID_CH], dt.bfloat16, name="z_slice")
            z_full = dram.tile([NPAD, HID_CH], dt.bfloat16, name="z_full",
                               addr_space="Shared")

            w1_t = cpool.tile([P, K1 * HID_CH], dt.bfloat16, name="w1_t")
            for m in range(K1):
                nc.sync.dma_start(out=w1_t[:, m * HID_CH:(m + 1) * HID_CH],
                                  in_=t_w1[m])
            w2_t = cpool.tile([P, K2 * HID_CH], dt.bfloat16, name="w2_t")
            for m in range(K2):
                nc.sync.dma_start(out=w2_t[:, m * HID_CH:(m + 1) * HID_CH],
                                  in_=t_w2[m])
            if has_bias:
                b1_t = cpool.tile([P, HID_CH], dt.float32, name="b1_t")
                nc.sync.dma_start(out=b1_t[:], in_=t_b1[:])
                b2_t = cpool.tile([P, HID_CH], dt.float32, name="b2_t")
                nc.sync.dma_start(out=b2_t[:], in_=t_b2[:])
            ident = cpool.tile([P, P], dt.float32, name="ident")
            make_identity(nc, ident[:])

            st_res = []  # S^T kept resident in SBUF; reused by L2
            # ---------------- Layer 1 ----------------
            with tc.tile_pool(name="psum1", bufs=2, space="PSUM") as psum1:
                for s in range(SUP_PER_CORE):
                    xe_t = pool.tile([P, B * IN_CH], dt.bfloat16, tag="xe",
                                     name=f"xe{s}")
                    nc.sync.dma_start(out=xe_t[:], in_=t_xe[s])
                    st_t = stpool.tile([P, B * P], dt.bfloat16, tag=f"st{s}",
                                       name=f"st{s}")
                    nc.sync.dma_start(out=st_t[:], in_=t_st[s])
                    st_res.append(st_t)

                    # row-major aggregation: S^T stationary, gathered rows stream
                    ag1 = psum1.tile([P, IN_CH], dt.float32, tag="ag1",
                                     name=f"ag1_{s}")
                    for b in range(B):
                        nc.tensor.matmul(
                            out=ag1[:],
                            lhsT=st_t[:, b * P:(b + 1) * P],
                            rhs=xe_t[:, b * IN_CH:(b + 1) * IN_CH],
                            start=(b == 0),
                            stop=(b == B - 1),
                        )
                    ag1r = pool.tile([P, IN_CH], dt.float32, tag="ag1r",
                                     name=f"ag1r{s}")
                    nc.vector.tensor_copy(out=ag1r[:], in_=ag1[:])
                    a1s = pool.tile([P, K1 * P], dt.bfloat16, tag="a1s",
                                    name=f"a1s{s}")
                    for m in range(K1):
                        tp = psum1.tile([P, P], dt.float32, tag="tp1",
                                        name=f"tp1_{s}_{m}")
                        nc.tensor.transpose(tp[:], ag1r[:, m * P:(m + 1) * P],
                                            ident[:])
                        nc.vector.tensor_copy(out=a1s[:, m * P:(m + 1) * P],
                                              in_=tp[:])

                    zp = psum1.tile([P, HID_CH], dt.float32, tag="zp",
                                    name=f"zp{s}")
                    for m in range(K1):
                        nc.tensor.matmul(
                            out=zp[:],
                            lhsT=a1s[:, m * P:(m + 1) * P],
                            rhs=w1_t[:, m * HID_CH:(m + 1) * HID_CH],
                            start=(m == 0),
                            stop=(m == K1 - 1),
                        )
                    z_t = pool.tile([P, HID_CH], dt.bfloat16, tag="z",
                                    name=f"z{s}")
                    if has_bias:
                        nc.vector.tensor_add(out=zp[:], in0=zp[:], in1=b1_t[:])
                    nc.scalar.activation(out=z_t[:], in_=zp[:],
                                         func=mybir.ActivationFunctionType.Relu)
                    nc.sync.dma_start(out=z_slice[s * P:(s + 1) * P, :], in_=z_t[:])

            # ---------------- AllGather z ----------------
            nc.gpsimd.collective_compute(
                "AllGather",
                mybir.AluOpType.bypass,
                replica_groups=[list(range(N_CORES))],
                ins=[z_slice.opt()],
                outs=[z_full.opt()],
            )
            if DEBUG_Z:
                nc.sync.dma_start(out=t_zdbg[:], in_=z_full[:])

            # ---------------- Layer 2 ----------------
            with tc.tile_pool(name="psum2", bufs=2, space="PSUM") as psum2:
                for s in range(SUP_PER_CORE):
                    zidx_t = pool.tile([P, B], dt.int32, tag="zidx",
                                       name=f"zidx{s}")
                    nc.sync.dma_start(out=zidx_t[:], in_=t_zidx[s])
                    g_t = pool.tile([P, B * HID_CH], dt.bfloat16, tag="g",
                                    name=f"g{s}")
                    for b in range(B):
                        nc.gpsimd.indirect_dma_start(
                            out=g_t[:, b * HID_CH:(b + 1) * HID_CH],
                            out_offset=None,
                            in_=z_full[:],
                            in_offset=bass.IndirectOffsetOnAxis(
                                ap=zidx_t[:, b:b + 1], axis=0
                            ),
                        )
                    st_t = st_res[s]
                    ag2 = psum2.tile([P, HID_CH], dt.float32, tag="ag2",
                                     name=f"ag2_{s}")
                    for b in range(B):
                        nc.tensor.matmul(
                            out=ag2[:],
                            lhsT=st_t[:, b * P:(b + 1) * P],
                            rhs=g_t[:, b * HID_CH:(b + 1) * HID_CH],
                            start=(b == 0),
                            stop=(b == B - 1),
                        )
                    ag2r = pool.tile([P, HID_CH], dt.float32, tag="ag2r",
                                     name=f"ag2r{s}")
                    nc.vector.tensor_copy(out=ag2r[:], in_=ag2[:])
                    a2s = pool.tile([P, K2 * P], dt.bfloat16, tag="a2s",
                                    name=f"a2s{s}")
                    for m in range(K2):
                        tp = psum2.tile([P, P], dt.float32, tag="tp2",
                                        name=f"tp2_{s}_{m}")
                        nc.tensor.transpose(tp[:], ag2r[:, m * P:(m + 1) * P],
                                            ident[:])
                        nc.vector.tensor_copy(out=a2s[:, m * P:(m + 1) * P],
                                              in_=tp[:])

                    op = psum2.tile([P, HID_CH], dt.float32, tag="op",
                                    name=f"op{s}")
                    for m in range(K2):
                        nc.tensor.matmul(
                            out=op[:],
                            lhsT=a2s[:, m * P:(m + 1) * P],
                            rhs=w2_t[:, m * HID_CH:(m + 1) * HID_CH],
                            start=(m == 0),
                            stop=(m == K2 - 1),
                        )
                    o_t = pool.tile([P, HID_CH], dt.float32, tag="o",
                                    name=f"o{s}")
                    if has_bias:
                        nc.vector.tensor_add(out=o_t[:], in0=op[:], in1=b2_t[:])
                    else:
                        nc.vector.tensor_copy(out=o_t[:], in_=op[:])
                    nc.sync.dma_start(out=t_out[s * P:(s + 1) * P, :], in_=o_t[:])

    nc.compile()
    return nc


def kernel(x, edge_index, W1, b1, W2, b2):
    global LAST_RESULTS
    from concourse import bass_utils

    x = np.asarray(x, np.float32)
    edge_index = np.asarray(edge_index)
    W1 = np.asarray(W1, np.float32)
    b1 = np.asarray(b1, np.float32)
    W2 = np.asarray(W2, np.float32)
    b2 = np.asarray(b2, np.float32)

    prep = _preprocess(x, edge_index)
    B = prep["B"]
    has_bias = bool(np.any(b1) or np.any(b2))

    key = (B, has_bias, DEBUG_Z)
    if key not in _COMPILED:
        _COMPILED[key] = _build_program(B, has_bias)
    nc = _COMPILED[key]

    w1_in = np.ascontiguousarray(
        W1.astype(bf16).reshape(2, P, HID_CH)
    )
    w2_in = np.ascontiguousarray(
        W2.astype(bf16).reshape(4, P, HID_CH)
    )

    in_maps = []
    for c in range(N_CORES):
        s0, s1 = c * SUP_PER_CORE, (c + 1) * SUP_PER_CORE
        m = {
            "xe": np.ascontiguousarray(prep["xe"][s0:s1]),
            "st": np.ascontiguousarray(prep["st"][s0:s1]),
            "zidx": np.ascontiguousarray(prep["zidx"][s0:s1]),
            "w1": w1_in,
            "w2": w2_in,
        }
        if has_bias:
            m["b1b"] = np.tile(b1.astype(np.float32)[None, :], (P, 1))
            m["b2b"] = np.tile(b2.astype(np.float32)[None, :], (P, 1))
        in_maps.append(m)

    res = bass_utils.run_bass_kernel_spmd(
        nc, in_maps, core_ids=list(range(N_CORES)), trace=TRACE,
    )
    LAST_RESULTS = res

    out = np.concatenate([res.results[c]["out"] for c in range(N_CORES)], axis=0)
    return np.ascontiguousarray(out[:N_NODES]).astype(np.float32)
